# revision 1
# baseline (speedup 1.0000x reference)
"""BasicConvClassifier on 8 Trainium2 NeuronCores.

Strategy (subject-sharded data parallelism):
  - Sort the batch by subject (4 subjects). Subject s's samples go to the core
    pair (2s, 2s+1); each core gets K samples (K = max over subjects of
    ceil(count/2), rounded up to a multiple of 8), short cores padded with
    zero samples.
  - Per-subject BatchNorm stats then only need a 2-rank AllReduce between the
    cores of a pair (per layer: 1KB payload), and all real samples on a core
    share one (scale, shift) pair, so the gelu apply is batched 8 samples per
    instruction.
  - Pad samples follow the exact same trajectory as a dedicated reference-pad
    slot (X = 0); their contribution to the statistics is subtracted exactly
    as npad * ref_contribution.
  - Conv1d(k=3, SAME) is shifted fp32r matmuls accumulated in PSUM; the
    residual of the H->H convs is folded into the center tap (W += I), and
    conv biases are dropped entirely (they cancel inside BatchNorm). Conv1's
    271 input channels x 3 taps are packed into 7 matmul passes (the 15
    leftover channels are pre-shifted on the host so all 3 taps contract in
    one pass).
  - Per layer: convs (PE) -> batched evict PSUM->SBUF (ACT/DVE, 4 samples per
    instruction) -> bn_stats in 512-element chunks spanning samples (DVE) ->
    decode partial sums, pairwise AllReduce, scale/shift -> batched gelu
    apply in place (ACT).
  - Head: time-mean pooling via per-group DVE reduces; pooled @ headW[:, :128]
    on PE; the per-subject constant (headW[:,128:] @ emb[s] + headb) is added
    during host-side unsharding.
"""

import numpy as np

_CACHE = {}

N_CORES = 8
CIN = 271
T = 281
TP = 284  # padded time: col 0 zero, cols 1..281 data, cols 282..283 zero
H = 128
S = 4
NCLS = 1854
NCHUNKS = (NCLS + 127) // 128  # 15
EPS = 1e-5
GS = 8  # samples per slot group
_EVOFF = 60  # evict priority boost (instructions)
CHUNK_SIZES = [512, 512, 512, 512, 224]  # per-group bn_stats chunking of 8*284


def _build(K):
    import concourse.bacc as bacc
    import concourse.tile as tile
    import concourse.mybir as mybir

    f32 = mybir.dt.float32
    f32r = mybir.dt.float32r
    AF = mybir.ActivationFunctionType
    OP = mybir.AluOpType
    AX = mybir.AxisListType

    W = K // GS
    NCH = 5 * W          # main bn_stats chunks
    assert sum(CHUNK_SIZES) == GS * TP

    nc = bacc.Bacc("TRN2", target_bir_lowering=False, num_devices=N_CORES)

    # ---- DRAM I/O ----
    bf16 = mybir.dt.bfloat16
    Xd = nc.dram_tensor("Xd", [K, 301, TP], bf16, kind="ExternalInput")
    W1d = nc.dram_tensor("W1d", [128, 7 * 128], bf16, kind="ExternalInput")
    WRd = nc.dram_tensor("WRd", [128, 24 * 128], f32r, kind="ExternalInput")
    GAMd = nc.dram_tensor("GAMd", [128, 9], f32, kind="ExternalInput")
    BETd = nc.dram_tensor("BETd", [128, 9], f32, kind="ExternalInput")
    INVCd = nc.dram_tensor("INVCd", [128, 1], f32, kind="ExternalInput")
    NPADd = nc.dram_tensor("NPADd", [128, 1], f32, kind="ExternalInput")
    CECd = nc.dram_tensor("CECd", [128, NCH], f32, kind="ExternalInput")
    EMBHd = nc.dram_tensor("EMBHd", [128, NCHUNKS * 128], f32r, kind="ExternalInput")
    OUTd = nc.dram_tensor("OUTd", [NCLS, K], f32, kind="ExternalOutput")

    cc_in = [nc.dram_tensor(f"ccin{l}", [128, 2], f32) for l in range(9)]
    cc_out = [nc.dram_tensor(f"ccout{l}", [2, 128, 2], f32) for l in range(9)]
    groups = [[0, 1], [2, 3], [4, 5], [6, 7]]

    with tile.TileContext(nc) as tc:
        # ---- static SBUF ----
        W1s = nc.alloc_sbuf_tensor("W1s", [128, 7 * 128], bf16)
        WRs = nc.alloc_sbuf_tensor("WRs", [128, 24 * 128], f32r)
        GAMs = nc.alloc_sbuf_tensor("GAMs", [128, 9], f32)
        BETs = nc.alloc_sbuf_tensor("BETs", [128, 9], f32)
        INVCs = nc.alloc_sbuf_tensor("INVCs", [128, 1], f32)
        NPADs = nc.alloc_sbuf_tensor("NPADs", [128, 1], f32)
        CECs = nc.alloc_sbuf_tensor("CECs", [128, NCH], f32)
        EMBHs = nc.alloc_sbuf_tensor("EMBHs", [128, NCHUNKS * 128], f32r)
        BNSTs = nc.alloc_sbuf_tensor("BNSTs", [128, (NCH + 1) * 6], f32)
        dA = nc.alloc_sbuf_tensor("dA", [128, NCH], f32)
        dB = nc.alloc_sbuf_tensor("dB", [128, NCH], f32)
        dC = nc.alloc_sbuf_tensor("dC", [128, NCH], f32)
        SST = nc.alloc_sbuf_tensor("SST", [128, 2], f32)
        SG = nc.alloc_sbuf_tensor("SG", [128, 2], f32)
        SG2 = nc.alloc_sbuf_tensor("SG2", [128, 4], f32)
        sm = [nc.alloc_sbuf_tensor(f"sm{i}", [128, 1], f32) for i in range(10)]
        EPSs = nc.alloc_sbuf_tensor("EPSs", [128, 1], f32)
        ZBIG = nc.alloc_sbuf_tensor("ZBIG", [128, TP], f32)
        P0 = nc.alloc_sbuf_tensor("P0", [128, K], f32)
        P0r = nc.alloc_sbuf_tensor("P0r", [128, K], f32r)
        ysg = [nc.alloc_sbuf_tensor(f"ysg{w}", [128, GS * TP], f32r)
               for w in range(W)]
        ysr = nc.alloc_sbuf_tensor("ysr", [128, TP], f32r)

        with tc.tile_pool(name="xpool", bufs=4) as xpool, \
             tc.tile_pool(name="xpool2", bufs=2) as xpool2, \
             tc.tile_pool(name="hcpool", bufs=8) as hcpool:

            # constant loads
            nc.sync.dma_start(out=W1s.ap(), in_=W1d.ap())
            nc.sync.dma_start(out=WRs.ap(), in_=WRd.ap())
            nc.sync.dma_start(out=GAMs.ap(), in_=GAMd.ap())
            nc.sync.dma_start(out=BETs.ap(), in_=BETd.ap())
            nc.sync.dma_start(out=INVCs.ap(), in_=INVCd.ap())
            nc.sync.dma_start(out=NPADs.ap(), in_=NPADd.ap())
            nc.sync.dma_start(out=CECs.ap(), in_=CECd.ap())
            nc.sync.dma_start(out=EMBHs.ap(), in_=EMBHd.ap())
            nc.gpsimd.memset(EPSs.ap(), EPS)
            nc.gpsimd.memset(ZBIG.ap(), 0.0)
            # zero the pad columns {0, 282, 283} of every slot
            for w in range(W):
                y3 = ysg[w].ap().rearrange("p (j t) -> p j t", j=GS)
                nc.gpsimd.tensor_copy(
                    out=y3[:, :, 0:1],
                    in_=ZBIG.ap()[:, 0:GS].rearrange("p (j o) -> p j o", o=1))
                nc.gpsimd.tensor_copy(
                    out=y3[:, :, 282:284],
                    in_=ZBIG.ap()[:, 0:2 * GS].rearrange("p (j o) -> p j o", o=2))
            nc.gpsimd.tensor_copy(out=ysr.ap()[:, 0:1], in_=ZBIG.ap()[:, 0:1])
            nc.gpsimd.tensor_copy(out=ysr.ap()[:, 282:284], in_=ZBIG.ap()[:, 0:2])

            def stats_and_sync(l):
                nc.scalar.activation(out=sm[2].ap(), in_=EPSs.ap(), func=AF.Sqrt,
                                     bias=EPSs.ap())
                bn3 = BNSTs.ap().rearrange("p (c s) -> p c s", s=6)
                ME, MO = bn3[:, 0:NCH, 1], bn3[:, 0:NCH, 4]
                CVE, CVO = bn3[:, 0:NCH, 2], bn3[:, 0:NCH, 5]
                # s1 = sum CEC*(ME+MO); s2 = sum CVE+CVO+CEC*(ME^2+MO^2)
                nc.gpsimd.tensor_tensor(out=dA.ap(), in0=ME, in1=MO, op=OP.add)
                nc.gpsimd.tensor_tensor(out=dA.ap(), in0=dA.ap(), in1=CECs.ap(), op=OP.mult)
                nc.vector.tensor_reduce(out=SST.ap()[:, 0:1], in_=dA.ap(),
                                        axis=AX.X, op=OP.add)
                nc.gpsimd.tensor_tensor(out=dB.ap(), in0=ME, in1=ME, op=OP.mult)
                nc.gpsimd.tensor_tensor(out=dC.ap(), in0=MO, in1=MO, op=OP.mult)
                nc.gpsimd.tensor_tensor(out=dB.ap(), in0=dB.ap(), in1=dC.ap(), op=OP.add)
                nc.gpsimd.tensor_tensor(out=dB.ap(), in0=dB.ap(), in1=CECs.ap(), op=OP.mult)
                nc.gpsimd.tensor_tensor(out=dB.ap(), in0=dB.ap(), in1=CVE, op=OP.add)
                nc.gpsimd.tensor_tensor(out=dB.ap(), in0=dB.ap(), in1=CVO, op=OP.add)
                nc.vector.tensor_reduce(out=SST.ap()[:, 1:2], in_=dB.ap(),
                                        axis=AX.X, op=OP.add)
                nc.sync.dma_start(out=cc_in[l].ap(), in_=SST.ap())
                # ref-pad contribution (identical on both pair cores): decoded
                # while the AllGather is in flight, subtracted post-gather as
                # npadsum * ref
                rb = 6 * NCH
                MEr = BNSTs.ap()[:, rb + 1:rb + 2]
                MOr = BNSTs.ap()[:, rb + 4:rb + 5]
                CVEr = BNSTs.ap()[:, rb + 2:rb + 3]
                CVOr = BNSTs.ap()[:, rb + 5:rb + 6]
                s1r, s2r, t0 = sm[0], sm[1], sm[2]
                nc.vector.tensor_tensor(out=s1r.ap(), in0=MEr, in1=MOr, op=OP.add)
                nc.vector.tensor_scalar(out=s1r.ap(), in0=s1r.ap(), scalar1=float(TP // 2),
                                        scalar2=None, op0=OP.mult)
                nc.vector.tensor_tensor(out=s2r.ap(), in0=MEr, in1=MEr, op=OP.mult)
                nc.vector.tensor_tensor(out=t0.ap(), in0=MOr, in1=MOr, op=OP.mult)
                nc.vector.tensor_tensor(out=s2r.ap(), in0=s2r.ap(), in1=t0.ap(), op=OP.add)
                nc.vector.tensor_scalar(out=s2r.ap(), in0=s2r.ap(), scalar1=float(TP // 2),
                                        scalar2=None, op0=OP.mult)
                nc.vector.tensor_tensor(out=s2r.ap(), in0=s2r.ap(), in1=CVEr, op=OP.add)
                nc.vector.tensor_tensor(out=s2r.ap(), in0=s2r.ap(), in1=CVOr, op=OP.add)
                nc.vector.tensor_tensor(out=s1r.ap(), in0=s1r.ap(), in1=NPADs.ap(), op=OP.mult)
                nc.vector.tensor_tensor(out=s2r.ap(), in0=s2r.ap(), in1=NPADs.ap(), op=OP.mult)
                nc.gpsimd.collective_compute(
                    "AllGather", OP.bypass, replica_groups=groups,
                    ins=[cc_in[l].ap()], outs=[cc_out[l].ap()])
                nc.sync.dma_start(
                    out=SG2.ap().rearrange("p (g s) -> p g s", g=2),
                    in_=cc_out[l].ap().rearrange("g p s -> p g s"))
                sg2 = SG2.ap().rearrange("p (g s) -> p g s", g=2)
                nc.vector.tensor_tensor(out=SG.ap(), in0=sg2[:, 0, :],
                                        in1=sg2[:, 1, :], op=OP.add)
                nc.vector.tensor_tensor(out=SG.ap()[:, 0:1], in0=SG.ap()[:, 0:1],
                                        in1=s1r.ap(), op=OP.subtract)
                nc.vector.tensor_tensor(out=SG.ap()[:, 1:2], in0=SG.ap()[:, 1:2],
                                        in1=s2r.ap(), op=OP.subtract)
                meanv, msqv, varv, sdv, invv, sclv, sftv = (
                    sm[3], sm[4], sm[5], sm[6], sm[7], sm[8], sm[9])
                nc.vector.tensor_tensor(out=meanv.ap(), in0=SG.ap()[:, 0:1],
                                        in1=INVCs.ap(), op=OP.mult)
                nc.vector.tensor_tensor(out=msqv.ap(), in0=SG.ap()[:, 1:2],
                                        in1=INVCs.ap(), op=OP.mult)
                nc.vector.tensor_tensor(out=varv.ap(), in0=meanv.ap(),
                                        in1=meanv.ap(), op=OP.mult)
                nc.vector.tensor_tensor(out=varv.ap(), in0=msqv.ap(),
                                        in1=varv.ap(), op=OP.subtract)
                nc.scalar.activation(out=sdv.ap(), in_=varv.ap(), func=AF.Sqrt,
                                     bias=EPSs.ap())
                nc.scalar.activation(out=sm[2].ap(), in_=EPSs.ap(), func=AF.Gelu,
                                     bias=EPSs.ap())
                nc.vector.reciprocal(out=invv.ap(), in_=sdv.ap())
                nc.vector.tensor_tensor(out=sclv.ap(), in0=GAMs.ap()[:, l:l + 1],
                                        in1=invv.ap(), op=OP.mult)
                nc.vector.tensor_tensor(out=sftv.ap(), in0=sclv.ap(),
                                        in1=meanv.ap(), op=OP.mult)
                nc.vector.tensor_tensor(out=sftv.ap(), in0=BETs.ap()[:, l:l + 1],
                                        in1=sftv.ap(), op=OP.subtract)
                return sclv, sftv

            def evict_and_stats(w, pss, l):
                # evict 2x 4-sample psum tiles into the group slab, then
                # bn_stats chunks over the slab
                y3 = ysg[w].ap().rearrange("p (j t) -> p j t", j=GS)
                for half, ps in enumerate(pss):
                    src = ps[:].rearrange("p (j t) -> p j t", j=4)[:, :, 0:T]
                    dst = y3[:, 4 * half:4 * half + 4, 1:1 + T]
                    with tc.high_priority(offset=_EVOFF):
                        if w < 6:
                            nc.vector.tensor_copy(out=dst, in_=src)
                        else:
                            nc.scalar.activation(out=dst, in_=src, func=AF.Copy)
                off = 0
                for i, csz in enumerate(CHUNK_SIZES):
                    c = 5 * w + i
                    nc.vector.bn_stats(out=BNSTs.ap()[:, 6 * c:6 * c + 6],
                                       in_=ysg[w].ap()[:, off:off + csz])
                    off += csz

            def ref_stats():
                nc.vector.bn_stats(out=BNSTs.ap()[:, 6 * NCH:6 * NCH + 6],
                                   in_=ysr.ap())

            def applies(l, sclv, sftv):
                for w in range(W):
                    y3 = ysg[w].ap().rearrange("p (j t) -> p j t", j=GS)
                    nc.scalar.activation(out=y3[:, :, 1:1 + T], in_=y3[:, :, 1:1 + T],
                                         func=AF.Gelu, bias=sftv.ap(),
                                         scale=sclv.ap())
                nc.scalar.activation(out=ysr.ap()[:, 1:1 + T], in_=ysr.ap()[:, 1:1 + T],
                                     func=AF.Gelu, bias=sftv.ap(), scale=sclv.ap())

            # ================= layer 0 (conv1: 271 -> 128) =================
            with tc.tile_pool(name="pspool", bufs=2, space="PSUM") as pspool:
                for w in range(W):
                    pss = []
                    for half in range(2):
                        ps = pspool.tile([128, 2048], f32, tag="ps")
                        pss.append(ps)
                        for j2 in range(2):
                            b = GS * w + 4 * half + 2 * j2
                            x0 = xpool.tile([128, 2, TP], bf16, tag="xc0")
                            x1 = xpool.tile([128, 2, TP], bf16, tag="xc1")
                            x2 = xpool2.tile([45, 2, TP], bf16, tag="xc2")
                            nc.sync.dma_start(out=x0[:], in_=Xd.ap()[b:b + 2, 0:128, :].rearrange("b c t -> c b t"))
                            nc.sync.dma_start(out=x1[:], in_=Xd.ap()[b:b + 2, 128:256, :].rearrange("b c t -> c b t"))
                            nc.sync.dma_start(out=x2[:], in_=Xd.ap()[b:b + 2, 256:301, :].rearrange("b c t -> c b t"))
                            for jj in range(2):
                                o = 512 * (2 * j2 + jj)
                                idx = 0
                                for xt, base in ((x0, 0), (x1, 3)):
                                    for k in range(3):
                                        nc.tensor.matmul(
                                            ps[:, o:o + 282],
                                            W1s.ap()[:, (base + k) * 128:(base + k + 1) * 128],
                                            xt[:, jj, k:k + 282],
                                            start=(idx == 0), stop=False)
                                        idx += 1
                                nc.tensor.matmul(
                                    ps[:, o:o + 282],
                                    W1s.ap()[0:45, 6 * 128:7 * 128],
                                    x2[0:45, jj, 1:283], start=False, stop=True)
                    evict_and_stats(w, pss, 0)
                # ref slot: conv(0) == 0
                nc.scalar.activation(out=ysr.ap()[:, 1:1 + T],
                                     in_=ZBIG.ap()[:, 0:T], func=AF.Copy)
                ref_stats()
                sclv, sftv = stats_and_sync(0)
                applies(0, sclv, sftv)

                # ================= layers 1..8 =================
                for l in range(1, 9):
                    w0 = (l - 1) * 3
                    for w in range(W):
                        pss = []
                        for half in range(2):
                            ps = pspool.tile([128, 2048], f32, tag="ps")
                            pss.append(ps)
                            for j4 in range(4):
                                j = 4 * half + j4
                                o = 512 * j4
                                for k in range(3):
                                    nc.tensor.matmul(
                                        ps[:, o:o + 282],
                                        WRs.ap()[:, (w0 + k) * 128:(w0 + k + 1) * 128],
                                        ysg[w].ap()[:, j * TP + k:j * TP + k + 282],
                                        start=(k == 0), stop=(k == 2))
                        evict_and_stats(w, pss, l)
                    # ref slot conv
                    psr = pspool.tile([128, 2048], f32, tag="ps")
                    for k in range(3):
                        nc.tensor.matmul(
                            psr[:, 0:282],
                            WRs.ap()[:, (w0 + k) * 128:(w0 + k + 1) * 128],
                            ysr.ap()[:, k:k + 282],
                            start=(k == 0), stop=(k == 2))
                    nc.scalar.activation(out=ysr.ap()[:, 1:1 + T],
                                         in_=psr[:, 0:T], func=AF.Copy)
                    ref_stats()
                    sclv, sftv = stats_and_sync(l)
                    applies(l, sclv, sftv)

                # pooling: P0[:, b] = sum_t ysg (post layer-8 gelu)
                for w in range(W):
                    y3 = ysg[w].ap().rearrange("p (j t) -> p j t", j=GS)
                    p03 = P0.ap().rearrange("p (k o) -> p k o", o=1)
                    with tc.high_priority():
                        nc.vector.tensor_reduce(
                            out=p03[:, GS * w:GS * w + GS, :],
                            in_=y3[:, :, 1:1 + T], axis=AX.X, op=OP.add)
                nc.vector.tensor_copy(out=P0r.ap(), in_=P0.ap())

            # ================= head =================
            with tc.tile_pool(name="hppool", bufs=8, space="PSUM") as hppool:
                for n in range(NCHUNKS):
                    rows = min(128, NCLS - n * 128)
                    hp = hppool.tile([128, K], f32, tag="hp")
                    nc.tensor.matmul(hp[:], EMBHs.ap()[:, n * 128:(n + 1) * 128],
                                     P0r.ap(), start=True, stop=True)
                    hc = hcpool.tile([128, K], f32, tag="hc")
                    nc.scalar.activation(out=hc[:], in_=hp[:], func=AF.Copy)
                    nc.sync.dma_start(out=OUTd.ap()[n * 128:n * 128 + rows, :],
                                      in_=hc[0:rows, :])

    nc.finalize()
    return nc


def kernel(**inputs):
    from concourse.bass_utils import run_bass_kernel_spmd

    X = np.asarray(inputs["X"], dtype=np.float32)
    w1_0 = np.asarray(inputs["w1_0"], dtype=np.float32)
    w_rest = np.asarray(inputs["w_rest"], dtype=np.float32)
    gammas = np.asarray(inputs["gammas"], dtype=np.float32)
    betas = np.asarray(inputs["betas"], dtype=np.float32)
    emb = np.asarray(inputs["emb"], dtype=np.float32)
    headW = np.asarray(inputs["headW"], dtype=np.float32)
    headb = np.asarray(inputs["headb"], dtype=np.float32)
    sidx = np.asarray(inputs["subject_idxs"]).astype(np.int64)

    B = X.shape[0]
    counts = np.bincount(sidx, minlength=S)
    K = int(max(GS, -(-counts.max() // 2)))
    K = ((K + GS - 1) // GS) * GS

    order = np.argsort(sidx, kind="stable")
    offs = np.zeros(S + 1, np.int64)
    offs[1:] = np.cumsum(counts)
    core_idxs = []
    for s in range(S):
        ids = order[offs[s]:offs[s + 1]]
        c0 = (len(ids) + 1) // 2
        core_idxs.append(ids[:c0])
        core_idxs.append(ids[c0:])

    # ---- shared host-side weight prep ----
    W1p = np.zeros((128, 7 * 128), np.float32)
    for c in range(2):
        for k in range(3):
            W1p[:, (c * 3 + k) * 128:(c * 3 + k + 1) * 128] = \
                w1_0[:, c * 128:(c + 1) * 128, k].T
    for k in range(3):
        W1p[15 * k:15 * k + 15, 6 * 128:7 * 128] = w1_0[:, 256:271, k].T
    WRp = np.zeros((128, 24 * 128), np.float32)
    eye = np.eye(H, dtype=np.float32)
    for l in range(8):
        for k in range(3):
            wt = w_rest[l, :, :, k].T.copy()
            if k == 1:
                wt += eye
            WRp[:, (l * 3 + k) * 128:(l * 3 + k + 1) * 128] = wt
    EMBHp = np.zeros((128, NCHUNKS * 128), np.float32)
    EMBHp[:, 0:NCLS] = headW[:, 0:H].T / float(T)
    Wg = K // GS
    CECp = np.tile(np.array([[c // 2 for c in CHUNK_SIZES]], np.float32),
                   (128, Wg)).astype(np.float32)

    in_maps = []
    for c in range(N_CORES):
        s = c // 2
        ids = core_idxs[c]
        n = len(ids)
        Xc = np.zeros((K, 301, TP), np.float32)
        if n:
            Xc[:n, 0:CIN, 1:1 + T] = X[ids]
            # pre-shifted 15-channel tail bands: band k at rows 256+15k..+15,
            # Xc[b, 256+15k+i, c] = xpad[b, 256+i, c+k-1]
            xt = Xc[:n, 256:CIN, :].copy()
            Xc[:n, 256:271, 1:] = xt[:, :, :-1]
            Xc[:n, 256:271, 0] = 0.0
            Xc[:n, 271:286, :] = xt
            Xc[:n, 286:301, :-1] = xt[:, :, 1:]
            Xc[:n, 286:301, -1] = 0.0
        INVC = np.full((128, 1), 1.0 / (max(int(counts[s]), 1) * T), np.float32)
        pair = [2 * s, 2 * s + 1]
        npadsum = sum(K - len(core_idxs[cc]) for cc in pair)
        NPAD = np.full((128, 1), float(npadsum), np.float32)
        import ml_dtypes
        in_maps.append({
            "Xd": Xc.astype(ml_dtypes.bfloat16),
            "W1d": W1p.astype(ml_dtypes.bfloat16),
            "WRd": WRp,
            "GAMd": gammas[:, s, :].T.copy(),
            "BETd": betas[:, s, :].T.copy(),
            "INVCd": INVC,
            "NPADd": NPAD,
            "CECd": CECp,
            "EMBHd": EMBHp,
        })

    if K not in _CACHE:
        _CACHE[K] = _build(K)
    nc = _CACHE[K]

    res = run_bass_kernel_spmd(nc, in_maps, core_ids=list(range(N_CORES)))
    kernel.last_results = res

    out = np.zeros((B, NCLS), np.float32)
    b2 = emb @ headW[:, H:].T + headb[None, :]  # [S, NCLS]
    for c in range(N_CORES):
        ids = core_idxs[c]
        if len(ids):
            out[ids] = res.results[c]["OUTd"].T[:len(ids)] + b2[c // 2][None, :]
    return out

